# revision 1
# baseline (speedup 1.0000x reference)
"""Trainium2 Bass kernel for the MERITS_T patient model (B=1024 data-parallel over 8 cores).

Mathematical simplification of the reference (verified to ~4e-7 rel err):
  - E_de = _mha(drug_mem, e0, e0) softmaxes over a single key, so its output is
    e0 @ m2_wv @ m2_wo broadcast over all 145 query rows -> the three GATs, the
    graph MHA and drug_mem never reach the output (dead code).
  - e0 = E_en[:, 0] only needs query row 0 of the m1 attention, i.e. only the
    first visit of `med`.
  - Per-head attention is refactored as u_h = mr0 @ (wq_h wk_h^T / sqrt(dh)),
    s_j = u_h . patient_j, r = sum_h (softmax-weighted patient avg) @ (wv_h wo_h m2_wv m2_wo).
  - final reshape tiles r 145x, so relu(final) @ out_w1 = relu(r) @ sum_m out_w1[m].
    The 43MB sum over m is sharded 8 ways and AllReduced on-device.

Per-core work (128 patients): static MLP over lab, glu/med encoders, one-query
attention over 25 visits, final MLP [64]->[1160]->[145].
"""

import numpy as np

import concourse.bass as bass
import concourse.mybir as mybir
from concourse.bass_utils import run_bass_kernel_spmd
from concourse.tile import TileContext

F32 = mybir.dt.float32
AF = mybir.ActivationFunctionType
ALU = mybir.AluOpType
AX = mybir.AxisListType

def split_multi_waits(nc):
    """The walrus on this image encodes at most ONE sync wait per TPB
    instruction ("Too many sync wait commands" otherwise). Hoist excess waits
    onto standalone InstEventSemaphore ops on the same engine, immediately
    before the instruction — the same mechanism Tile's barriers use."""
    wid = 0
    for f in nc.m.functions:
        for bb in f.blocks:
            out = []
            for ins in bb.instructions:
                si = ins.sync_info
                if si is not None and si.on_wait and len(si.on_wait) > 1:
                    waits = list(si.on_wait)
                    for w in waits[:-1]:
                        wid += 1
                        out.append(mybir.InstEventSemaphore(
                            name=f"Wsplit-{wid}", engine=ins.engine,
                            ins=[], outs=[],
                            sync_info=mybir.SyncInfo(on_wait=[w], on_update=[])))
                    si.on_wait = waits[-1:]
                out.append(ins)
            bb.instructions = out
    return wid


B, T, MED, LAB, GLU, D, H = 1024, 25, 145, 1956, 16, 64, 32
NC_CORES = 8
BC = B // NC_CORES  # 128 patients per core
NH, DH = 4, 16
HID = MED * D // 8  # 1160
MBLK = 19  # ceil(145/8) out_w1 blocks per core (zero-padded)


def build_bass(split_waits=True):
    nc = bass.Bass()

    # ---- I/O declarations (per-core shapes) ----
    def inp(name, shape):
        return nc.dram_tensor(name, list(shape), F32, kind="ExternalInput")

    lab_d = inp("lab", (BC, LAB + 1))  # ones column appended (bias fold)
    glu_d = inp("glu", (BC, T, GLU))
    tf_d = inp("tf", (BC, T, GLU))
    med_d = inp("med", (BC, T, MED))
    w1s_d = inp("w1shard", (MBLK, D, HID))
    sllw1_d = inp("sll_w1", (LAB + 1, D))  # bias row appended
    sllw2_d = inp("sll_w2", (D + 1, H))    # bias row appended
    gluw_d = inp("glu_w", (2 * GLU, H))
    glub_d = inp("glu_b", (1, H))
    glug_d = inp("glu_gate", (1, H))
    medw_d = inp("med_w", (MED + 1, D))    # bias row appended
    medg_d = inp("med_gate", (1, D))
    wq_d = inp("m1_wq", (D, D))
    wk_d = inp("m1_wk", (D, D))
    wv_d = inp("m1_wv", (D, D))
    wo_d = inp("m1_wo", (D, D))
    m2wv_d = inp("m2_wv", (D, D))
    m2wo_d = inp("m2_wo", (D, D))
    outb1_d = inp("out_b1", (1, HID))
    outw2_d = inp("out_w2", (HID + 1, MED))  # bias row appended
    out_d = nc.dram_tensor("out", [BC, MED], F32, kind="ExternalOutput")

    # Inline constants: per-head partition masks and the glu block-diag mask
    hm = np.zeros((D, NH), np.float32)
    for h in range(NH):
        hm[h * DH:(h + 1) * DH, h] = 1.0
    hmask_d = nc.inline_tensor(hm, name="head_mask")
    bdm = np.zeros((128, 8 * H), np.float32)
    for jl in range(8):
        bdm[jl * GLU:(jl + 1) * GLU, jl * H:(jl + 1) * H] = 1.0
    bdmask_d = nc.inline_tensor(bdm, name="bd_mask")
    ident_d = nc.inline_tensor(np.eye(128, dtype=np.float32), name="ident128")

    # Internal DRAM for the W1s AllReduce (kept in the [128, 580] layout the
    # on-chip reduce produces; the [64, 1160] regather is a linear DRAM view)
    cc_in = nc.dram_tensor("cc_in", [128, 580], F32)
    cc_out = nc.dram_tensor("cc_out", [128, 580], F32, addr_space="Shared")

    with TileContext(nc) as tc, \
            tc.tile_pool(name="consts", bufs=1) as cp, \
            tc.tile_pool(name="ps", bufs=2, space="PSUM") as ps, \
            tc.tile_pool(name="psg", bufs=1, space="PSUM") as psg:

        dma = nc.sync.dma_start

        # ================= W1s shard sum + AllReduce (long latency, start early) ====
        # shard viewed as [(f h)=128 partitions, m=19, i=580]
        w1v = w1s_d[:].rearrange("m f (h i) -> (f h) m i", h=2)
        w1raw = cp.tile([128, MBLK, 580], F32, tag="w1raw")
        for q in range(4):
            dma(out=w1raw[:, :, q * 145:(q + 1) * 145],
                in_=w1v[:, :, q * 145:(q + 1) * 145])
        w1red = cp.tile([128, 580], F32, tag="w1red")
        for q in range(4):  # one reduce per DMA chunk (ISA sync-wait limit)
            nc.vector.tensor_reduce(
                out=w1red[:, q * 145:(q + 1) * 145],
                in_=w1raw[:, :, q * 145:(q + 1) * 145].rearrange("p m i -> p i m"),
                axis=AX.X, op=ALU.add)
        dma(out=cc_in[:], in_=w1red[:])
        nc.gpsimd.collective_compute(
            "AllReduce", ALU.add, replica_groups=[list(range(NC_CORES))],
            ins=[cc_in[:]], outs=[cc_out[:]])
        w1s_sb = cp.tile([D + 1, HID], F32, tag="w1s_sb")
        dma(out=w1s_sb[0:D, :], in_=cc_out[:].rearrange("(f h) i -> f (h i)", h=2))
        dma(out=w1s_sb[D:D + 1, :], in_=outb1_d[:])

        # ================= constants / weights =====================================
        ident = cp.tile([128, 128], F32, tag="ident")
        dma(out=ident, in_=ident_d[:])

        # sll_w1 tiles [128, 16, 64]; host already appended the bias row
        w1sb = cp.tile([128, 16, D], F32, tag="w1sb")
        dma(out=w1sb[:, 0:15, :], in_=sllw1_d[0:1920, :].rearrange("(t k) d -> k t d", k=128))
        dma(out=w1sb[0:37, 15, :], in_=sllw1_d[1920:1957, :])
        w2sb = cp.tile([D + 1, H], F32, tag="w2sb")
        dma(out=w2sb, in_=sllw2_d[:])
        gw_g3 = cp.tile([GLU, H], F32, tag="gw_g3")
        dma(out=gw_g3, in_=gluw_d[0:GLU, :])
        gw_t3 = cp.tile([GLU, H], F32, tag="gw_t3")
        dma(out=gw_t3, in_=gluw_d[GLU:2 * GLU, :])
        # block-diagonal glu weights: one broadcast DMA replicates glu_w[rows]
        # into every (jl, jl') block of [128, 256]; an inline 0/1 mask then
        # zeroes the off-diagonal blocks. One K=128 matmul projects 8 visits.
        bdmask = cp.tile([128, 8 * H], F32, tag="bdmask")
        dma(out=bdmask, in_=bdmask_d[:])

        def build_wbd(row0, tag):
            rep = cp.tile([128, H], F32, tag=tag + "_rep")
            dma(out=rep,
                in_=gluw_d[row0:row0 + GLU, :].unsqueeze(0).broadcast_to((8, GLU, H)))
            wbd = cp.tile([128, 8, H], F32, tag=tag)
            nc.vector.tensor_mul(wbd,
                                 rep[:].unsqueeze(1).broadcast_to((128, 8, H)),
                                 bdmask[:].rearrange("p (j o) -> p j o", j=8))
            return wbd

        wbd_g = build_wbd(0, "wbd_g")
        wbd_t = build_wbd(GLU, "wbd_t")
        gbb = cp.tile([128, H], F32, tag="gbb")
        dma(out=gbb, in_=glub_d[:].broadcast_to((128, H)))
        ggb = cp.tile([128, H], F32, tag="ggb")
        dma(out=ggb, in_=glug_d[:].broadcast_to((128, H)))
        mwsb = cp.tile([128, D], F32, tag="mwsb")
        dma(out=mwsb, in_=medw_d[0:128, :])
        mw2sb = cp.tile([18, D], F32, tag="mw2sb")
        dma(out=mw2sb, in_=medw_d[128:146, :])
        mgb = cp.tile([128, D], F32, tag="mgb")
        dma(out=mgb, in_=medg_d[:].broadcast_to((128, D)))
        ow2sb = cp.tile([128, 10, MED], F32, tag="ow2sb")
        dma(out=ow2sb[:, 0:9, :], in_=outw2_d[0:1152, :].rearrange("(t k) n -> k t n", k=128))
        dma(out=ow2sb[0:9, 9, :], in_=outw2_d[1152:1161, :])

        wq_sb = cp.tile([D, D], F32, tag="wq_sb")
        dma(out=wq_sb, in_=wq_d[:])
        wk_sb = cp.tile([D, D], F32, tag="wk_sb")
        dma(out=wk_sb, in_=wk_d[:])
        wv_sb = cp.tile([D, D], F32, tag="wv_sb")
        dma(out=wv_sb, in_=wv_d[:])
        wo_sb = cp.tile([D, D], F32, tag="wo_sb")
        dma(out=wo_sb, in_=wo_d[:])
        m2wv_sb = cp.tile([D, D], F32, tag="m2wv_sb")
        dma(out=m2wv_sb, in_=m2wv_d[:])
        m2wo_sb = cp.tile([D, D], F32, tag="m2wo_sb")
        dma(out=m2wo_sb, in_=m2wo_d[:])

        # ---- weight prep on PE: wqT/wkT/wvT/m2wvT, A_h, M_hT, Wvo2, MW_h ----
        def transpose_to_sbuf(src_ap, rows, cols, sb_tile, copy_engine=None):
            pt = ps.tile([cols, rows], F32, tag="tp")
            nc.tensor.transpose(pt[0:cols, 0:rows], src_ap, ident[0:rows, 0:rows])
            copy_op = nc.scalar.copy if copy_engine is None else nc.vector.tensor_copy
            copy_op(out=sb_tile[0:cols, 0:rows], in_=pt[0:cols, 0:rows])

        wqT = cp.tile([D, D], F32, tag="wqT")
        transpose_to_sbuf(wq_sb[:], D, D, wqT)
        wkT = cp.tile([D, D], F32, tag="wkT")
        transpose_to_sbuf(wk_sb[:], D, D, wkT)
        wvT = cp.tile([D, D], F32, tag="wvT")
        transpose_to_sbuf(wv_sb[:], D, D, wvT)
        m2wvT = cp.tile([D, D], F32, tag="m2wvT")
        transpose_to_sbuf(m2wv_sb[:], D, D, m2wvT)

        # head-masked copies of wkT / wvT (rows outside head h zeroed) so the
        # per-head products contract over the full K=64 at base partition 0
        hmask = cp.tile([D, NH], F32, tag="hmask")
        dma(out=hmask, in_=hmask_d[:])
        wkT4 = cp.tile([D, NH, D], F32, tag="wkT4")
        wvT4 = cp.tile([D, NH, D], F32, tag="wvT4")
        for h in range(NH):
            nc.vector.tensor_scalar(out=wkT4[:, h, :], in0=wkT[:],
                                    scalar1=hmask[:, h:h + 1], scalar2=None,
                                    op0=ALU.mult)
            nc.vector.tensor_scalar(out=wvT4[:, h, :], in0=wvT[:],
                                    scalar1=hmask[:, h:h + 1], scalar2=None,
                                    op0=ALU.mult)
        # A_h = wq_h @ wk_h^T / 4
        a_ps = ps.tile([D, NH, D], F32, tag="acc")
        for h in range(NH):
            nc.tensor.matmul(a_ps[:, h, :], lhsT=wqT[:], rhs=wkT4[:, h, :])
        ah_sb = cp.tile([D, NH, D], F32, tag="ah_sb")
        nc.scalar.activation(out=ah_sb, in_=a_ps, func=AF.Copy, scale=1.0 / np.sqrt(DH))
        # M_hT[e,f] = (wv_h @ wo_h)^T
        m_ps = ps.tile([D, NH, D], F32, tag="acc")
        for h in range(NH):
            nc.tensor.matmul(m_ps[:, h, :], lhsT=wo_sb[:], rhs=wvT4[:, h, :])
        mhT_sb = cp.tile([D, NH, D], F32, tag="mhT_sb")
        nc.scalar.copy(out=mhT_sb, in_=m_ps)
        # Wvo2 = m2_wv @ m2_wo
        wvo_ps = ps.tile([D, D], F32, tag="acc")
        nc.tensor.matmul(wvo_ps, lhsT=m2wvT[:], rhs=m2wo_sb[:])
        wvo_sb = cp.tile([D, D], F32, tag="wvo_sb")
        nc.scalar.copy(out=wvo_sb, in_=wvo_ps)
        # MW_h = M_h @ Wvo2, then stacked vertically: mw_stack[h*64+f, e']
        mw_ps = ps.tile([D, NH, D], F32, tag="acc")
        for h in range(NH):
            nc.tensor.matmul(mw_ps[:, h, :], lhsT=mhT_sb[:, h, :], rhs=wvo_sb[:])
        mw_sb = cp.tile([D, NH, D], F32, tag="mw_sb")
        nc.scalar.copy(out=mw_sb, in_=mw_ps)
        mw_stack = cp.tile([128, 2, D], F32, tag="mw_stack")
        for h in range(NH):
            dma(out=mw_stack[(h % 2) * D:(h % 2 + 1) * D, h // 2, :],
                in_=mw_sb[:, h, :])

        # ================= static MLP over lab =====================================
        lab_sb = cp.tile([128, LAB + 1], F32, tag="lab_sb")
        dma(out=lab_sb[:, 0:1024], in_=lab_d[:, 0:1024])
        dma(out=lab_sb[:, 1024:LAB + 1], in_=lab_d[:, 1024:LAB + 1])
        labT = cp.tile([128, 16, 128], F32, tag="labT")
        for g in range(4):
            pt = ps.tile([128, 4, 128], F32, tag="grp")
            for i in range(4):
                kt = 4 * g + i
                w = 128 if kt < 15 else 37
                nc.tensor.transpose(pt[0:w, i, :], lab_sb[:, kt * 128:kt * 128 + w],
                                    ident[:])
            if g < 3:
                nc.vector.tensor_copy(out=labT[:, 4 * g:4 * g + 4, :], in_=pt[:])
            else:
                nc.vector.tensor_copy(out=labT[:, 12:15, :], in_=pt[:, 0:3, :])
                nc.vector.tensor_copy(out=labT[0:37, 15, :], in_=pt[0:37, 3, :])

        st1_ps = ps.tile([128, D], F32, tag="acc")
        for kt in range(16):
            k = 128 if kt < 15 else 37
            nc.tensor.matmul(st1_ps, lhsT=labT[0:k, kt, :], rhs=w1sb[0:k, kt, :],
                             start=(kt == 0), stop=(kt == 15))
        st1r = cp.tile([128, D], F32, tag="st1r")
        nc.scalar.activation(out=st1r, in_=st1_ps, func=AF.Relu)
        st1rT = cp.tile([D + 1, 128], F32, tag="st1rT")
        transpose_to_sbuf(st1r[:], 128, D, st1rT)
        nc.vector.memset(st1rT[D:D + 1, :], 1.0)
        st2_ps = ps.tile([128, H], F32, tag="acc")
        nc.tensor.matmul(st2_ps, lhsT=st1rT[:], rhs=w2sb[:])
        static_sb = cp.tile([128, H], F32, tag="static_sb")
        nc.scalar.activation(out=static_sb, in_=st2_ps, func=AF.Relu)

        # ================= glu encoder =============================================
        glu_sb = cp.tile([128, T * GLU], F32, tag="glu_sb")
        dma(out=glu_sb, in_=glu_d[:].rearrange("p j f -> p (j f)"))
        tf_sb = cp.tile([128, T * GLU], F32, tag="tf_sb")
        dma(out=tf_sb, in_=tf_d[:].rearrange("p j f -> p (j f)"))

        def transpose400(src, dst_tag, copy_op):
            pt = ps.tile([128, 4, 128], F32, tag="grp")
            for c in range(4):
                w = 128 if c < 3 else 16
                nc.tensor.transpose(pt[0:w, c, :], src[:, c * 128:c * 128 + w],
                                    ident[:])
            dst = cp.tile([128, 4, 128], F32, tag=dst_tag)
            copy_op(out=dst[:, 0:3, :], in_=pt[:, 0:3, :])
            copy_op(out=dst[0:16, 3, :], in_=pt[0:16, 3, :])
            return dst

        gluT = transpose400(glu_sb, "gluT", nc.scalar.copy)
        tfT = transpose400(tf_sb, "tfT", nc.vector.tensor_copy)

        patient = cp.tile([128, T, D], F32, tag="patient")
        # block-diagonal projection: chunk c of the (j,f)-major transpose covers
        # visits 8c..8c+7; one K=128 matmul against wbd projects all 8 at once
        gx_ps = psg.tile([128, T, H], F32, tag="gx")
        for c in range(3):
            nc.tensor.matmul(gx_ps[:, 8 * c:8 * c + 8, :], lhsT=gluT[:, c, :],
                             rhs=wbd_g[:], start=True, stop=False)
            nc.tensor.matmul(gx_ps[:, 8 * c:8 * c + 8, :], lhsT=tfT[:, c, :],
                             rhs=wbd_t[:], start=False, stop=True)
        nc.tensor.matmul(gx_ps[:, 24, :], lhsT=gluT[0:GLU, 3, :],
                         rhs=gw_g3[:], start=True, stop=False)
        nc.tensor.matmul(gx_ps[:, 24, :], lhsT=tfT[0:GLU, 3, :],
                         rhs=gw_t3[:], start=False, stop=True)
        gxb = cp.tile([128, T, H], F32, tag="gxb")
        nc.vector.tensor_add(gxb, gx_ps,
                             gbb[:].unsqueeze(1).broadcast_to((128, T, H)))
        nc.scalar.activation(out=patient[:, :, 0:H], in_=gxb, func=AF.Tanh)
        # gate = sigmoid(gx . glu_gate); patient[:, :, :H] *= gate
        gm = cp.tile([128, T, H], F32, tag="gm")
        nc.vector.tensor_mul(gm, patient[:, :, 0:H],
                             ggb[:].unsqueeze(1).broadcast_to((128, T, H)))
        gs = cp.tile([128, T], F32, tag="gs")
        nc.vector.tensor_reduce(out=gs, in_=gm, axis=AX.X, op=ALU.add)
        gsg = cp.tile([128, T], F32, tag="gsg")
        nc.scalar.activation(out=gsg, in_=gs, func=AF.Sigmoid)
        nc.vector.tensor_mul(patient[:, :, 0:H], patient[:, :, 0:H],
                             gsg[:].unsqueeze(2).broadcast_to((128, T, H)))
        # static broadcast into patient[:, :, H:D]
        nc.vector.tensor_copy(out=patient[:, :, H:D],
                              in_=static_sb[:].unsqueeze(1).broadcast_to((128, T, H)))

        # ================= med first-visit encoder =================================
        med0 = cp.tile([128, MED], F32, tag="med0")
        dma(out=med0, in_=med_d[:, 0, :])
        mb = cp.tile([128, MED + 1], F32, tag="mb")
        nc.vector.tensor_scalar(out=mb[:, 0:MED], in0=med0, scalar1=0.9,
                                scalar2=None, op0=ALU.is_gt)
        nc.vector.memset(mb[:, MED:MED + 1], 1.0)  # ones column (bias fold)
        mbTa = cp.tile([128, 128], F32, tag="mbTa")
        transpose_to_sbuf(mb[:, 0:128], 128, 128, mbTa, copy_engine=nc.vector)
        mbTb = cp.tile([18, 128], F32, tag="mbTb")
        transpose_to_sbuf(mb[:, 128:146], 128, 18, mbTb, copy_engine=nc.vector)
        x0_ps = ps.tile([128, D], F32, tag="acc")
        nc.tensor.matmul(x0_ps, lhsT=mbTa[:], rhs=mwsb[:], start=True, stop=False)
        nc.tensor.matmul(x0_ps, lhsT=mbTb[:], rhs=mw2sb[:], start=False, stop=True)
        x0 = cp.tile([128, D], F32, tag="x0")
        nc.vector.tensor_copy(out=x0, in_=x0_ps)
        scr = cp.tile([128, D], F32, tag="scr")
        nc.vector.tensor_mul(scr, x0, mgb)
        g0 = cp.tile([128, 1], F32, tag="g0")
        nc.vector.tensor_reduce(out=g0, in_=scr, axis=AX.X, op=ALU.add)
        sg0 = cp.tile([128, 1], F32, tag="sg0")
        nc.scalar.activation(out=sg0, in_=g0, func=AF.Sigmoid)
        mr0 = cp.tile([128, D], F32, tag="mr0")
        nc.vector.tensor_scalar(out=mr0, in0=x0, scalar1=sg0[:, 0:1], scalar2=None,
                                op0=ALU.mult)
        mr0T = cp.tile([D, 128], F32, tag="mr0T")
        transpose_to_sbuf(mr0[:], 128, D, mr0T, copy_engine=nc.vector)

        # ================= one-query attention =====================================
        u_ps = ps.tile([128, NH, D], F32, tag="acc")
        for h in range(NH):
            nc.tensor.matmul(u_ps[:, h, :], lhsT=mr0T[:], rhs=ah_sb[:, h, :])
        u_sb = cp.tile([128, NH, D], F32, tag="u_sb")
        nc.vector.tensor_copy(out=u_sb, in_=u_ps)

        sprod = cp.tile([128, T, NH, D], F32, tag="bigscratch")
        nc.vector.tensor_mul(sprod,
                             patient[:].unsqueeze(2).broadcast_to((128, T, NH, D)),
                             u_sb[:].unsqueeze(1).broadcast_to((128, T, NH, D)))
        s_sb = cp.tile([128, T, NH], F32, tag="s_sb")
        nc.vector.tensor_reduce(out=s_sb, in_=sprod, axis=AX.X, op=ALU.add)
        es = cp.tile([128, T, NH], F32, tag="es")
        nc.scalar.activation(out=es, in_=s_sb, func=AF.Exp)
        den = cp.tile([128, NH], F32, tag="den")
        nc.vector.tensor_reduce(out=den, in_=es.rearrange("p j h -> p h j"),
                                axis=AX.X, op=ALU.add)
        rden = cp.tile([128, NH], F32, tag="rden")
        nc.vector.reciprocal(out=rden, in_=den)
        attn = cp.tile([128, T, NH], F32, tag="attn")
        nc.vector.tensor_mul(attn, es, rden[:].unsqueeze(1).broadcast_to((128, T, NH)))

        wprod = cp.tile([128, NH, T, D], F32, tag="bigscratch2")
        nc.vector.tensor_mul(
            wprod,
            attn.rearrange("p j h -> p h j").unsqueeze(3).broadcast_to((128, NH, T, D)),
            patient[:].unsqueeze(1).broadcast_to((128, NH, T, D)))
        w_sb = cp.tile([128, NH, D], F32, tag="w_sb")
        nc.vector.tensor_reduce(out=w_sb, in_=wprod.rearrange("p h j f -> p h f j"),
                                axis=AX.X, op=ALU.add)

        wT = cp.tile([128, 2, 128], F32, tag="wT")
        wflat = w_sb.rearrange("p h f -> p (h f)")
        for c in range(2):
            pt = ps.tile([128, 128], F32, tag="tp")
            nc.tensor.transpose(pt[:], wflat[:, c * 128:(c + 1) * 128], ident[:])
            nc.vector.tensor_copy(out=wT[:, c, :], in_=pt[:])

        r_ps = ps.tile([128, D], F32, tag="acc")
        for c in range(2):
            nc.tensor.matmul(r_ps, lhsT=wT[:, c, :], rhs=mw_stack[:, c, :],
                             start=(c == 0), stop=(c == 1))
        rr = cp.tile([128, D], F32, tag="rr")
        nc.scalar.activation(out=rr, in_=r_ps, func=AF.Relu)
        rrT = cp.tile([D + 1, 128], F32, tag="rrT")
        transpose_to_sbuf(rr[:], 128, D, rrT, copy_engine=nc.vector)
        nc.vector.memset(rrT[D:D + 1, :], 1.0)

        # ================= final MLP ===============================================
        hid = cp.tile([128, HID + 1], F32, tag="hid")
        for o, n in [(0, 512), (512, 512), (1024, 136)]:
            h_ps = ps.tile([128, 512], F32, tag="acc")
            nc.tensor.matmul(h_ps[:, 0:n], lhsT=rrT[:], rhs=w1s_sb[:, o:o + n])
            nc.scalar.activation(out=hid[:, o:o + n], in_=h_ps[:, 0:n], func=AF.Relu)
        nc.vector.memset(hid[:, HID:HID + 1], 1.0)  # ones column (bias fold)
        hidT = cp.tile([128, 10, 128], F32, tag="hidT")
        for g in range(3):
            n_in_g = 4 if g < 2 else 2
            pt = ps.tile([128, 4, 128], F32, tag="grp")
            for i in range(n_in_g):
                kt = 4 * g + i
                w = 128 if kt < 9 else 9
                nc.tensor.transpose(pt[0:w, i, :], hid[:, kt * 128:kt * 128 + w],
                                    ident[:])
            if g < 2:
                nc.vector.tensor_copy(out=hidT[:, 4 * g:4 * g + 4, :], in_=pt[:])
            else:
                nc.vector.tensor_copy(out=hidT[:, 8:9, :], in_=pt[:, 0:1, :])
                nc.vector.tensor_copy(out=hidT[0:9, 9, :], in_=pt[0:9, 1, :])

        out_ps = ps.tile([128, MED], F32, tag="acc")
        for kt in range(10):
            k = 128 if kt < 9 else 9
            nc.tensor.matmul(out_ps, lhsT=hidT[0:k, kt, :], rhs=ow2sb[0:k, kt, :],
                             start=(kt == 0), stop=(kt == 9))
        out_sb = cp.tile([128, MED], F32, tag="out_sb")
        nc.vector.tensor_copy(out=out_sb, in_=out_ps)
        dma(out=out_d[:], in_=out_sb)

    if split_waits:
        split_multi_waits(nc)
    return nc


_CACHED_NC = None


def make_in_maps(inputs):
    f = lambda x: np.ascontiguousarray(np.asarray(x, dtype=np.float32))
    # out_w1 blocks [145, 64, 1160] -> 8 zero-padded shards of 19 blocks
    w1blocks = f(inputs["out_w1"]).reshape(MED, D, HID)
    shards = np.zeros((NC_CORES, MBLK, D, HID), np.float32)
    flat = np.zeros((NC_CORES * MBLK, D, HID), np.float32)
    flat[:MED] = w1blocks
    shards[:] = flat.reshape(NC_CORES, MBLK, D, HID)

    # host-side bias folding: append bias rows to weights / ones column to lab
    # (pure input marshalling; all arithmetic stays on device)
    cat = np.concatenate
    rep = {
        "sll_w1": cat([f(inputs["sll_w1"]), f(inputs["sll_b1"]).reshape(1, D)], 0),
        "sll_w2": cat([f(inputs["sll_w2"]), f(inputs["sll_b2"]).reshape(1, H)], 0),
        "glu_w": f(inputs["glu_w"]), "glu_b": f(inputs["glu_b"]).reshape(1, H),
        "glu_gate": f(inputs["glu_gate"]).reshape(1, H),
        "med_w": cat([f(inputs["med_w"]), f(inputs["med_b"]).reshape(1, D)], 0),
        "med_gate": f(inputs["med_gate"]).reshape(1, D),
        "m1_wq": f(inputs["m1_wq"]), "m1_wk": f(inputs["m1_wk"]),
        "m1_wv": f(inputs["m1_wv"]), "m1_wo": f(inputs["m1_wo"]),
        "m2_wv": f(inputs["m2_wv"]), "m2_wo": f(inputs["m2_wo"]),
        "out_b1": f(inputs["out_b1"]).reshape(1, HID),
        "out_w2": cat([f(inputs["out_w2"]), f(inputs["out_b2"]).reshape(1, MED)], 0),
    }
    lab = cat([f(inputs["lab"]), np.ones((B, 1), np.float32)], 1)
    glu, tf, med = f(inputs["glu"]), f(inputs["time_feat"]), f(inputs["med"])

    in_maps = []
    for c in range(NC_CORES):
        sl = slice(c * BC, (c + 1) * BC)
        in_maps.append({
            "lab": lab[sl], "glu": glu[sl], "tf": tf[sl], "med": med[sl],
            "w1shard": shards[c], **rep,
        })
    return in_maps


def kernel(**inputs):
    global _CACHED_NC
    if _CACHED_NC is None:
        _CACHED_NC = build_bass()
    nc = _CACHED_NC
    in_maps = make_in_maps(inputs)
    res = run_bass_kernel_spmd(nc, in_maps, core_ids=list(range(NC_CORES)))
    return np.concatenate([res.results[c]["out"] for c in range(NC_CORES)], axis=0)


if __name__ == "__main__":
    import reference
    inp = reference.setup_inputs()
    out = kernel(**{k: np.asarray(v) for k, v in inp.items()})
    print("kernel output", out.shape, out.dtype)



# revision 6
# speedup vs baseline: 1.3833x; 1.3833x over previous
"""Trainium2 Bass kernel for the MERITS_T patient model (B=1024 data-parallel over 8 cores).

Mathematical simplification of the reference (verified to ~4e-7 rel err):
  - E_de = _mha(drug_mem, e0, e0) softmaxes over a single key, so its output is
    e0 @ m2_wv @ m2_wo broadcast over all 145 query rows -> the three GATs, the
    graph MHA and drug_mem never reach the output (dead code).
  - e0 = E_en[:, 0] only needs query row 0 of the m1 attention, i.e. only the
    first visit of `med`.
  - patient_j = [glu_rep_j | static]: the static half is visit-independent, so
    it shifts all logits equally (softmax-invariant) and its attention-weighted
    average is just `static` (weights sum to 1). Attention therefore only runs
    on the 32-dim glu half; the static half re-enters linearly at the end via
    SS = sum_h MW_h[32:64] where MW_h = wv_h wo_h m2_wv m2_wo.
  - The gate sigma(x.glu_gate) multiplies logits and values linearly, so it is
    folded in as a scalar after the score reduce / into the softmax weights.
  - final reshape tiles r 145x, so relu(final) @ out_w1 = relu(r) @ sum_m out_w1[m].
    The 43MB sum over m is sharded 8 ways (bf16) and AllReduced on-device.

All host work is input marshalling only (transpose / reshape / concat / pad /
dtype cast); every arithmetic op runs on device. The dataflow is arranged so
each matmul produces its output pre-transposed for the next consumer; the only
on-device transposes are the four 32-row y_h tiles feeding the final r matmul.
"""

import numpy as np
import ml_dtypes

import concourse.bass as bass
import concourse.mybir as mybir
from concourse.bass_utils import run_bass_kernel_spmd
from concourse.tile import TileContext

F32 = mybir.dt.float32
BF16 = mybir.dt.bfloat16
AF = mybir.ActivationFunctionType
ALU = mybir.AluOpType
AX = mybir.AxisListType


def split_multi_waits(nc):
    """The walrus on this image encodes at most ONE sync wait per TPB
    instruction ("Too many sync wait commands" otherwise). Hoist excess waits
    onto standalone InstEventSemaphore ops on the same engine, immediately
    before the instruction — the same mechanism Tile's barriers use."""
    wid = 0
    for f in nc.m.functions:
        for bb in f.blocks:
            out = []
            for ins in bb.instructions:
                si = ins.sync_info
                if si is not None and si.on_wait and len(si.on_wait) > 1:
                    waits = list(si.on_wait)
                    for w in waits[:-1]:
                        wid += 1
                        out.append(mybir.InstEventSemaphore(
                            name=f"Wsplit-{wid}", engine=ins.engine,
                            ins=[], outs=[],
                            sync_info=mybir.SyncInfo(on_wait=[w], on_update=[])))
                    si.on_wait = waits[-1:]
                out.append(ins)
            bb.instructions = out
    return wid


B, T, MED, LAB, GLU, D, H = 1024, 25, 145, 1956, 16, 64, 32
NC_CORES = 8
BC = B // NC_CORES       # 128 patients per core
NH, DH = 4, 16
HID = MED * D // 8       # 1160
MBLK = 20                # 19 real out_w1 blocks per core + 1 zero pad (even for DVE 2x)
TP = T + 1               # visit dim padded to 26 (even) for the j-reduce
KLAB = 16                # 2048 = 16*128 lab contraction tiles (1956 + bias + pad)


def build_bass(split_waits=True):
    nc = bass.Bass()

    def inp(name, shape, dt=F32):
        return nc.dram_tensor(name, list(shape), dt, kind="ExternalInput")

    # ---- per-core inputs (host-marshalled layouts; see make_in_maps) ----
    labT_d = inp("labT", (KLAB * 128, BC))          # lab^T + ones row + zero pad
    gluT_d = inp("gluT", (512, BC))                 # glu  [(j f), p] zero-padded
    tfT_d = inp("tfT", (512, BC))                   # time_feat, same layout
    med0T_d = inp("med0T", (MED + 1, BC))           # med visit-0 ^T + ones row
    w1s_d = inp("w1shard", (128, 580, MBLK), BF16)  # out_w1 shard [(f h), i, m]
    w1sb_d = inp("w1sbH", (128, KLAB * D))          # sll_w1+b1 as [k, (t d)]
    w2sb_d = inp("w2sbH", (D + 1, H))               # sll_w2 + bias row
    gw3_d = inp("gw3H", (GLU, 2 * H))               # glu_w rows for visit 24 [glu|tf]
    gb8_d = inp("gb8H", (1, 8 * H))                 # glu_b tiled 8x
    ggb_d = inp("ggbH", (1, H))                     # glu_gate
    wbd_d = inp("wbdH", (128, 16 * H))              # block-diag glu_w [glu 8H | tf 8H]
    mwsb_d = inp("mwsbH", (128, D))                 # med_w rows 0..127
    mw2sb_d = inp("mw2sbH", (18, D))                # med_w rows 128..144 + med_b
    mgT_d = inp("mgTH", (D, 1))                     # med_gate column
    wqT4_d = inp("wqT4H", (DH, NH * D))             # wq^T per head [c, (h d)]
    wkT4_d = inp("wkT4H", (DH, NH * D))             # wk^T per head [c, (h f)]
    wvT4_d = inp("wvT4H", (DH, NH * D))             # wv^T per head [c, (h f)]
    woT_d = inp("woTH", (D, D))                     # wo^T
    m2wvT_d = inp("m2wvTH", (D, D))                 # m2_wv^T
    m2wo_d = inp("m2woH", (D, D))
    outb1_d = inp("outb1H", (1, HID), BF16)
    ow2sb_d = inp("ow2sbH", (128, 10 * MED))        # out_w2+b2 as [k, (t n)]
    out_d = nc.dram_tensor("out", [BC, MED], F32, kind="ExternalOutput")

    ident_d = nc.inline_tensor(np.eye(128, dtype=np.float32), name="ident128")

    # internal DRAM for the bf16 W1s AllReduce
    cc_in = nc.dram_tensor("cc_in", [128, 580], BF16)
    cc_out = nc.dram_tensor("cc_out", [128, 580], BF16, addr_space="Shared")

    with TileContext(nc) as tc, \
            tc.tile_pool(name="consts", bufs=1) as cp, \
            tc.tile_pool(name="ps", bufs=2, space="PSUM") as ps, \
            tc.tile_pool(name="pst", bufs=1, space="PSUM") as pst, \
            tc.tile_pool(name="psg", bufs=1, space="PSUM") as psg:

        dma = nc.sync.dma_start

        # ================= tiny weights first (weight-prep deps) ==============
        ident = cp.tile([128, 128], F32, tag="ident")
        dma(out=ident, in_=ident_d[:])
        wqT4 = cp.tile([DH, NH, D], F32, tag="wqT4")
        dma(out=wqT4, in_=wqT4_d[:].rearrange("c (h d) -> c h d", h=NH))
        wkT4 = cp.tile([DH, NH, D], F32, tag="wkT4")
        dma(out=wkT4, in_=wkT4_d[:].rearrange("c (h d) -> c h d", h=NH))
        wvT4 = cp.tile([DH, NH, D], F32, tag="wvT4")
        dma(out=wvT4, in_=wvT4_d[:].rearrange("c (h d) -> c h d", h=NH))
        woT = cp.tile([D, D], F32, tag="woT")
        dma(out=woT, in_=woT_d[:])
        m2wvT = cp.tile([D, D], F32, tag="m2wvT")
        dma(out=m2wvT, in_=m2wvT_d[:])
        m2wo = cp.tile([D, D], F32, tag="m2wo")
        dma(out=m2wo, in_=m2wo_d[:])
        mwsb = cp.tile([128, D], F32, tag="mwsb")
        dma(out=mwsb, in_=mwsb_d[:])
        mw2sb = cp.tile([18, D], F32, tag="mw2sb")
        dma(out=mw2sb, in_=mw2sb_d[:])
        mgT = cp.tile([D, 1], F32, tag="mgT")
        dma(out=mgT, in_=mgT_d[:])
        w2sb = cp.tile([D + 1, H], F32, tag="w2sb")
        dma(out=w2sb, in_=w2sb_d[:])
        gw3 = cp.tile([GLU, 2 * H], F32, tag="gw3")
        dma(out=gw3, in_=gw3_d[:])
        gb8 = cp.tile([1, 8 * H], F32, tag="gb8")
        dma(out=gb8, in_=gb8_d[:])
        ggb = cp.tile([128, H], F32, tag="ggb")
        dma(out=ggb, in_=ggb_d[:].broadcast_to((128, H)))
        wbd = cp.tile([128, 16, H], F32, tag="wbd")
        dma(out=wbd, in_=wbd_d[:].rearrange("k (t h) -> k t h", h=H))
        outb1 = cp.tile([1, HID], BF16, tag="outb1")
        dma(out=outb1, in_=outb1_d[:])

        ones1 = cp.tile([1, 128], F32, tag="ones1")
        nc.vector.memset(ones1, 1.0)

        # ================= W1s shard sum + AllReduce (start ASAP) ============
        w1raw = cp.tile([128, 580, MBLK], BF16, tag="w1raw")
        w1red = cp.tile([128, 580], F32, tag="w1red")
        for q in range(4):
            sl = slice(q * 145, (q + 1) * 145)
            dma(out=w1raw[:, sl, :], in_=w1s_d[:, sl, :])
            nc.vector.tensor_reduce(out=w1red[:, sl], in_=w1raw[:, sl, :],
                                    axis=AX.X, op=ALU.add)
        ccs = cp.tile([128, 580], BF16, tag="ccs")
        nc.vector.tensor_copy(out=ccs, in_=w1red)
        dma(out=cc_in[:], in_=ccs)
        nc.gpsimd.collective_compute(
            "AllReduce", ALU.add, replica_groups=[list(range(NC_CORES))],
            ins=[cc_in[:]], outs=[cc_out[:]])
        w1s_sb = cp.tile([D + 1, HID], BF16, tag="w1s_sb")
        dma(out=w1s_sb[0:D, :], in_=cc_out[:].rearrange("(f h) i -> f (h i)", h=2))
        dma(out=w1s_sb[D:D + 1, :], in_=outb1)

        # ================= bulk input DMAs ===================================
        med0Ta = cp.tile([128, BC], F32, tag="med0Ta")
        dma(out=med0Ta, in_=med0T_d[0:128, :])
        med0Tb = cp.tile([18, BC], F32, tag="med0Tb")
        dma(out=med0Tb, in_=med0T_d[128:MED + 1, :])
        gluT = cp.tile([128, 4, BC], F32, tag="gluT")
        dma(out=gluT, in_=gluT_d[:].rearrange("(c k) p -> k c p", k=128))
        tfT = cp.tile([128, 4, BC], F32, tag="tfT")
        dma(out=tfT, in_=tfT_d[:].rearrange("(c k) p -> k c p", k=128))
        labT = cp.tile([128, KLAB, BC], F32, tag="labT")
        dma(out=labT, in_=labT_d[:].rearrange("(t k) p -> k t p", k=128))
        w1sb = cp.tile([128, KLAB, D], F32, tag="w1sb")
        dma(out=w1sb, in_=w1sb_d[:].rearrange("k (t d) -> k t d", d=D))
        ow2sb = cp.tile([128, 10, MED], F32, tag="ow2sb")
        dma(out=ow2sb, in_=ow2sb_d[:].rearrange("k (t n) -> k t n", n=MED))

        # ================= weight prep on PE =================================
        # Wvo2 = m2_wv @ m2_wo
        wvo_ps = ps.tile([D, D], F32, tag="acc")
        nc.tensor.matmul(wvo_ps, lhsT=m2wvT[:], rhs=m2wo[:])
        wvo2 = cp.tile([D, D], F32, tag="wvo2")
        nc.scalar.copy(out=wvo2, in_=wvo_ps)
        # WoV_h = wo[h-rows] @ Wvo2, all heads -> [c, h, e]
        wov_ps = ps.tile([DH, NH, D], F32, tag="acc")
        for h in range(NH):
            nc.tensor.matmul(wov_ps[:, h, :], lhsT=woT[:, h * DH:(h + 1) * DH],
                             rhs=wvo2[:])
        wov4 = cp.tile([DH, NH, D], F32, tag="wov4")
        nc.scalar.copy(out=wov4, in_=wov_ps)
        # MWg_h = wv_h[0:32 rows] @ WoV_h  -> [f, h, e]; SS = sum_h wv_h[32:] @ WoV_h
        mw_ps = ps.tile([H, NH, D], F32, tag="acc")
        for h in range(NH):
            nc.tensor.matmul(mw_ps[:, h, :], lhsT=wvT4[:, h, 0:H],
                             rhs=wov4[:, h, :])
        mw4 = cp.tile([H, NH, D], F32, tag="mw4")
        nc.scalar.copy(out=mw4, in_=mw_ps)
        ss_ps = ps.tile([H, D], F32, tag="acc")
        for h in range(NH):
            nc.tensor.matmul(ss_ps, lhsT=wvT4[:, h, H:D], rhs=wov4[:, h, :],
                             start=(h == 0), stop=(h == NH - 1))
        ss_sb = cp.tile([H, D], F32, tag="ss_sb")
        nc.scalar.copy(out=ss_sb, in_=ss_ps)
        # A_h[:, 0:32]/4 stacked along free dim -> [d, (h f)]
        ahg_ps = ps.tile([D, NH, H], F32, tag="acc")
        for h in range(NH):
            nc.tensor.matmul(ahg_ps[:, h, :], lhsT=wqT4[:, h, :],
                             rhs=wkT4[:, h, 0:H])
        ahg = cp.tile([D, NH, H], F32, tag="ahg")
        nc.scalar.activation(out=ahg, in_=ahg_ps, func=AF.Copy, scale=1.0 / DH ** 0.5)

        # ================= med visit-0 encoder (all transposed) ==============
        mbTa = cp.tile([128, BC], F32, tag="mbTa")
        nc.vector.tensor_scalar(out=mbTa, in0=med0Ta, scalar1=0.9, scalar2=None,
                                op0=ALU.is_gt)
        mbTb = cp.tile([18, BC], F32, tag="mbTb")
        nc.vector.tensor_scalar(out=mbTb, in0=med0Tb, scalar1=0.9, scalar2=None,
                                op0=ALU.is_gt)
        x0_ps = ps.tile([D, BC], F32, tag="acc")
        nc.tensor.matmul(x0_ps, lhsT=mwsb[:], rhs=mbTa[:], start=True, stop=False)
        nc.tensor.matmul(x0_ps, lhsT=mw2sb[:], rhs=mbTb[:], start=False, stop=True)
        x0T = cp.tile([D, BC], F32, tag="x0T")
        nc.vector.tensor_copy(out=x0T, in_=x0_ps)
        g0_ps = ps.tile([1, BC], F32, tag="acc")
        nc.tensor.matmul(g0_ps, lhsT=mgT[:], rhs=x0T[:])
        sg0T = cp.tile([1, BC], F32, tag="sg0T")
        nc.scalar.activation(out=sg0T, in_=g0_ps, func=AF.Sigmoid)
        sg0r_ps = ps.tile([D, BC], F32, tag="acc")
        nc.tensor.matmul(sg0r_ps, lhsT=ones1[0:1, 0:D], rhs=sg0T[:])
        mr0T = cp.tile([D, BC], F32, tag="mr0T")
        nc.vector.tensor_mul(mr0T, x0T, sg0r_ps)
        # u_g[p, (h f)] = mr0 @ A_h[:, 0:32]
        u_ps = ps.tile([BC, NH, H], F32, tag="acc")
        nc.tensor.matmul(u_ps, lhsT=mr0T[:], rhs=ahg[:].rearrange("d h f -> d (h f)"))
        u_sb = cp.tile([BC, NH, H], F32, tag="u_sb")
        nc.vector.tensor_copy(out=u_sb, in_=u_ps)

        # ================= glu encoder x = tanh(glu_in @ glu_w + b) ==========
        gx_ps = psg.tile([128, T, H], F32, tag="gx")
        for c in range(3):
            sl8 = slice(8 * c, 8 * c + 8)
            nc.tensor.matmul(gx_ps[:, sl8, :], lhsT=gluT[:, c, :],
                             rhs=wbd[:, 0:8, :], start=True, stop=False)
            nc.tensor.matmul(gx_ps[:, sl8, :], lhsT=tfT[:, c, :],
                             rhs=wbd[:, 8:16, :], start=False, stop=False)
            nc.tensor.matmul(gx_ps[:, sl8, :], lhsT=ones1[0:1, :],
                             rhs=gb8[:], start=False, stop=True)
        nc.tensor.matmul(gx_ps[:, 24, :], lhsT=gluT[0:GLU, 3, :],
                         rhs=gw3[:, 0:H], start=True, stop=False)
        nc.tensor.matmul(gx_ps[:, 24, :], lhsT=tfT[0:GLU, 3, :],
                         rhs=gw3[:, H:2 * H], start=False, stop=False)
        nc.tensor.matmul(gx_ps[:, 24, :], lhsT=ones1[0:1, :],
                         rhs=gb8[0:1, 0:H], start=False, stop=True)
        x_sb = cp.tile([128, TP, H], F32, tag="x_sb")
        nc.vector.memset(x_sb[:, T, :], 0.0)
        nc.scalar.activation(out=x_sb[:, 0:T, :], in_=gx_ps, func=AF.Tanh)

        # gate = sigmoid(x . glu_gate)
        gm = cp.tile([128, T, H], F32, tag="gm")
        nc.vector.tensor_mul(gm, x_sb[:, 0:T, :],
                             ggb[:].unsqueeze(1).broadcast_to((128, T, H)))
        gs = cp.tile([128, T], F32, tag="gs")
        nc.vector.tensor_reduce(out=gs, in_=gm, axis=AX.X, op=ALU.add)
        gate = cp.tile([128, T], F32, tag="gate")
        nc.scalar.activation(out=gate, in_=gs, func=AF.Sigmoid)

        # ================= static MLP over lab (all transposed) ==============
        st1_ps = ps.tile([D, BC], F32, tag="acc")
        for t in range(KLAB):
            nc.tensor.matmul(st1_ps, lhsT=w1sb[:, t, :], rhs=labT[:, t, :],
                             start=(t == 0), stop=(t == KLAB - 1))
        st1rT = cp.tile([D + 1, BC], F32, tag="st1rT")
        nc.scalar.activation(out=st1rT[0:D, :], in_=st1_ps, func=AF.Relu)
        nc.vector.memset(st1rT[D:D + 1, :], 1.0)
        st2_ps = ps.tile([H, BC], F32, tag="acc")
        nc.tensor.matmul(st2_ps, lhsT=w2sb[:], rhs=st1rT[:])
        staticT = cp.tile([H, BC], F32, tag="staticT")
        nc.scalar.activation(out=staticT, in_=st2_ps, func=AF.Relu)

        # ================= one-query attention (glu half only) ===============
        sprod = cp.tile([128, T, NH, H], F32, tag="sprod")
        nc.vector.tensor_mul(
            sprod,
            x_sb[:, 0:T, :].unsqueeze(2).broadcast_to((128, T, NH, H)),
            u_sb[:].unsqueeze(1).broadcast_to((128, T, NH, H)))
        s_sb = cp.tile([128, T, NH], F32, tag="s_sb")
        nc.vector.tensor_reduce(out=s_sb, in_=sprod, axis=AX.X, op=ALU.add)
        sg_sb = cp.tile([128, T, NH], F32, tag="sg_sb")
        nc.vector.tensor_mul(sg_sb, s_sb,
                             gate[:].unsqueeze(2).broadcast_to((128, T, NH)))
        es = cp.tile([128, T, NH], F32, tag="es")
        nc.scalar.activation(out=es, in_=sg_sb, func=AF.Exp)
        den = cp.tile([128, NH], F32, tag="den")
        nc.vector.tensor_reduce(out=den, in_=es.rearrange("p j h -> p h j"),
                                axis=AX.X, op=ALU.add)
        rden = cp.tile([128, NH], F32, tag="rden")
        nc.vector.reciprocal(out=rden, in_=den)
        # coef = es * gate * rden  (softmax weight x value gate), j padded to 26
        cg = cp.tile([128, T, NH], F32, tag="cg")
        nc.vector.tensor_mul(cg, es, gate[:].unsqueeze(2).broadcast_to((128, T, NH)))
        coef = cp.tile([128, TP, NH], F32, tag="coef")
        nc.vector.memset(coef[:, T, :], 0.0)
        nc.vector.tensor_mul(coef[:, 0:T, :], cg,
                             rden[:].unsqueeze(1).broadcast_to((128, T, NH)))
        # y_glu[p, h, f] = sum_j coef * x  (j innermost for the reduce)
        wprod = cp.tile([128, NH, H, TP], F32, tag="wprod")
        nc.vector.tensor_mul(
            wprod,
            coef.rearrange("p j h -> p h j").unsqueeze(2).broadcast_to((128, NH, H, TP)),
            x_sb.rearrange("p j f -> p f j").unsqueeze(1).broadcast_to((128, NH, H, TP)))
        y_sb = cp.tile([128, NH, H], F32, tag="y_sb")
        nc.vector.tensor_reduce(out=y_sb, in_=wprod, axis=AX.X, op=ALU.add)

        # rT = sum_h MWg_h^T-free @ y_h^T + SS^T-free @ staticT
        yT4 = cp.tile([H, NH, BC], F32, tag="yT4")
        yt_ps = pst.tile([H, NH, BC], F32, tag="tp")
        for h in range(NH):
            nc.tensor.transpose(yt_ps[:, h, :], y_sb[:, h, :], ident[0:BC, 0:BC])
        nc.vector.tensor_copy(out=yT4, in_=yt_ps)
        rT_ps = ps.tile([D, BC], F32, tag="acc")
        for h in range(NH):
            nc.tensor.matmul(rT_ps, lhsT=mw4[:, h, :], rhs=yT4[:, h, :],
                             start=(h == 0), stop=False)
        nc.tensor.matmul(rT_ps, lhsT=ss_sb[:], rhs=staticT[:],
                         start=False, stop=True)
        rrT = cp.tile([D + 1, BC], F32, tag="rrT")
        nc.scalar.activation(out=rrT[0:D, :], in_=rT_ps, func=AF.Relu)
        nc.vector.memset(rrT[D:D + 1, :], 1.0)
        rrTb = cp.tile([D + 1, BC], BF16, tag="rrTb")
        nc.vector.tensor_copy(out=rrTb, in_=rrT)

        # ================= final MLP (transposed tail) =======================
        hidT = cp.tile([128, 10, BC], F32, tag="hidT")
        # ones everywhere in tile 9; the t=9 relu overwrites rows 0..7 and only
        # rows 0..8 (hid + bias row) feed the final matmul
        nc.vector.memset(hidT[:, 9, :], 1.0)
        for t in range(10):
            n = 128 if t < 9 else 8
            h_ps = ps.tile([128, BC], F32, tag="hacc")
            nc.tensor.matmul(h_ps[0:n, :], lhsT=w1s_sb[:, t * 128:t * 128 + n],
                             rhs=rrTb[:])
            if t % 2 == 0:
                nc.scalar.activation(out=hidT[0:n, t, :], in_=h_ps[0:n, :],
                                     func=AF.Relu)
            else:
                nc.vector.tensor_scalar(out=hidT[0:n, t, :], in0=h_ps[0:n, :],
                                        scalar1=0.0, scalar2=None, op0=ALU.max)
        out_ps = psg.tile([BC, MED], F32, tag="outp")
        for t in range(10):
            k = 128 if t < 9 else 9
            nc.tensor.matmul(out_ps, lhsT=hidT[0:k, t, :], rhs=ow2sb[0:k, t, :],
                             start=(t == 0), stop=(t == 9))
        out_sb = cp.tile([BC, MED], F32, tag="out_sb")
        nc.vector.tensor_copy(out=out_sb, in_=out_ps)
        dma(out=out_d[:], in_=out_sb)

    if split_waits:
        split_multi_waits(nc)
    return nc


_CACHED_NC = None


def make_in_maps(inputs):
    """Pure input marshalling: transpose / reshape / concat / pad / cast only."""
    f = lambda x: np.ascontiguousarray(np.asarray(x, dtype=np.float32))
    cat = np.concatenate
    bf = lambda x: np.ascontiguousarray(x.astype(ml_dtypes.bfloat16))

    lab = f(inputs["lab"])
    glu = f(inputs["glu"]).reshape(B, T * GLU)
    tf = f(inputs["time_feat"]).reshape(B, T * GLU)
    med0 = f(inputs["med"])[:, 0, :]

    # out_w1 -> per-core bf16 shards [(f h), i, m]
    w1b = f(inputs["out_w1"]).reshape(MED, D, HID)
    w1pad = np.zeros((NC_CORES * 19 + NC_CORES, D, HID), np.float32)  # 160 blocks
    # lay real blocks into per-core groups of 19 (20th stays zero)
    for c in range(NC_CORES):
        lo = c * 19
        n = min(19, MED - lo)
        if n > 0:
            w1pad[c * MBLK:c * MBLK + n] = w1b[lo:lo + n]
    shards = []
    for c in range(NC_CORES):
        blk = w1pad[c * MBLK:(c + 1) * MBLK]               # [20, 64, 1160]
        s = blk.reshape(MBLK, D, 2, 580).transpose(1, 2, 3, 0).reshape(128, 580, MBLK)
        shards.append(bf(s))

    # sll_w1 + bias, padded to 2048 rows, as [k, (t d)]
    w1cat = np.zeros((KLAB * 128, D), np.float32)
    w1cat[0:LAB] = f(inputs["sll_w1"])
    w1cat[LAB] = f(inputs["sll_b1"])
    w1sbH = np.ascontiguousarray(
        w1cat.reshape(KLAB, 128, D).transpose(1, 0, 2).reshape(128, KLAB * D))

    w2sbH = cat([f(inputs["sll_w2"]), f(inputs["sll_b2"]).reshape(1, H)], 0)

    glu_w = f(inputs["glu_w"])                              # [32, 32]
    gwg, gwt = glu_w[0:GLU], glu_w[GLU:2 * GLU]
    wbdH = np.zeros((128, 16 * H), np.float32)
    for jl in range(8):
        wbdH[jl * GLU:(jl + 1) * GLU, jl * H:(jl + 1) * H] = gwg
        wbdH[jl * GLU:(jl + 1) * GLU, (8 + jl) * H:(9 + jl) * H] = gwt
    gw3H = np.ascontiguousarray(cat([gwg, gwt], 1))         # [16, 64]
    gb8H = np.tile(f(inputs["glu_b"]).reshape(1, H), (1, 8))

    medw = f(inputs["med_w"])                               # [145, 64]
    mw2 = cat([medw[128:MED], f(inputs["med_b"]).reshape(1, D)], 0)  # [18, 64]

    def headT(w):  # [64, 64] -> [c, (h d)] with w^T per head block
        wt = f(w).T.reshape(NH, DH, D)                      # [h, c, d]
        return np.ascontiguousarray(wt.transpose(1, 0, 2).reshape(DH, NH * D))

    # out_w2 + bias, padded to 1280 rows, as [k, (t n)]
    w2cat = np.zeros((1280, MED), np.float32)
    w2cat[0:HID] = f(inputs["out_w2"])
    w2cat[HID] = f(inputs["out_b2"])
    ow2sbH = np.ascontiguousarray(
        w2cat.reshape(10, 128, MED).transpose(1, 0, 2).reshape(128, 10 * MED))

    rep = {
        "w1sbH": w1sbH, "w2sbH": w2sbH, "gw3H": gw3H, "gb8H": gb8H,
        "ggbH": f(inputs["glu_gate"]).reshape(1, H), "wbdH": wbdH,
        "mwsbH": np.ascontiguousarray(medw[0:128]), "mw2sbH": np.ascontiguousarray(mw2),
        "mgTH": f(inputs["med_gate"]).reshape(D, 1),
        "wqT4H": headT(inputs["m1_wq"]), "wkT4H": headT(inputs["m1_wk"]),
        "wvT4H": headT(inputs["m1_wv"]),
        "woTH": np.ascontiguousarray(f(inputs["m1_wo"]).T),
        "m2wvTH": np.ascontiguousarray(f(inputs["m2_wv"]).T),
        "m2woH": f(inputs["m2_wo"]),
        "outb1H": bf(f(inputs["out_b1"]).reshape(1, HID)),
        "ow2sbH": ow2sbH,
    }

    in_maps = []
    for c in range(NC_CORES):
        sl = slice(c * BC, (c + 1) * BC)
        labTc = np.zeros((KLAB * 128, BC), np.float32)
        labTc[0:LAB] = lab[sl].T
        labTc[LAB] = 1.0
        gluTc = np.zeros((512, BC), np.float32)
        gluTc[0:T * GLU] = glu[sl].T
        tfTc = np.zeros((512, BC), np.float32)
        tfTc[0:T * GLU] = tf[sl].T
        med0Tc = np.ones((MED + 1, BC), np.float32)
        med0Tc[0:MED] = med0[sl].T
        in_maps.append({
            "labT": labTc, "gluT": gluTc, "tfT": tfTc, "med0T": med0Tc,
            "w1shard": shards[c], **rep,
        })
    return in_maps


def kernel(**inputs):
    global _CACHED_NC
    if _CACHED_NC is None:
        _CACHED_NC = build_bass()
    nc = _CACHED_NC
    in_maps = make_in_maps(inputs)
    res = run_bass_kernel_spmd(nc, in_maps, core_ids=list(range(NC_CORES)))
    return np.concatenate([res.results[c]["out"] for c in range(NC_CORES)], axis=0)


if __name__ == "__main__":
    import reference
    inp = reference.setup_inputs()
    out = kernel(**{k: np.asarray(v) for k, v in inp.items()})
    print("kernel output", out.shape, out.dtype)
